# revision 3
# baseline (speedup 1.0000x reference)
"""Trainium2 Bass kernel for nn_Attention_49340584296826 (pointer-generator
attention with coverage).

Computation per batch row b (B=32, T=2048, N=1024):
    dec   = s_t_hat @ W_dec.T + b_dec                       [N]
    x     = ef[b] + dec[None,:] + coverage[b][:,None]*W_c   [T, N]
    e     = tanh(x)
    s     = e @ v_w                                         [T]
    a     = exp(s - S) * mask;  a /= sum(a)   (S = sum|v_w| >= max s)
    a    += stmt * mask
    c_t   = a @ encoder_outputs[b]                          [N]
    cov'  = coverage + a

Sharding: data-parallel over batch, 4 rows per core on 8 cores.

Layout: t-chunks of 512 rows per DMA, arranged [128 partitions, 4, N] with
t = 512*q + 4*p + a (16KB contiguous per partition). Scores live as
[128, 16] with column j = 4*q + a.

All large tensors ship as host-split hi/lo bf16 pairs (hi+lo reconstructs
fp32 to ~2^-17), so every big matmul runs at bf16 rate with ~1e-5 accuracy:
  PE  : psum = deccov (K=5 bf16: [1,1,covh,covh,covl] x [dech,decl,wch,wcl,wch])
        psum += ef    (identity matmuls against efh, efl)
        c_t accumulation (attn hi/lo x eo hi/lo, 3 passes)
  ACT : e = tanh(psum)
  DVE : affine_mul_reduce(e * v_w) -> scores column (fused mult+rowsum)
Softmax normalization uses PE ones-matmuls for cross-partition sum/broadcast.
"""
import numpy as np
from contextlib import ExitStack

import concourse.bass as bass
import concourse.bacc as bacc
import concourse.tile as tile
from concourse import mybir
from concourse.bass_utils import run_bass_kernel_spmd

F32 = mybir.dt.float32
BF16 = mybir.dt.bfloat16
AF = mybir.ActivationFunctionType
ALU = mybir.AluOpType
AX = mybir.AxisListType

B, T, N = 32, 2048, 1024
NCORES = 8
BL = B // NCORES          # 4 batch rows per core
P = 128
A = 4                     # t rows per partition per DMA chunk
QT = P * A                # 512 t per chunk
NQ = T // QT              # 4 chunks per batch
NJ = NQ * A               # 16 score columns per batch
H = 512                   # matmul free-dim split (one PSUM bank)
KC = N // P               # 8 contraction chunks for the dec matmul


def _bc(ap, parts):
    """Broadcast an AP across `parts` partitions (stride-0 partition dim)."""
    return bass.AP(tensor=ap.tensor, offset=ap.offset, ap=[[0, parts]] + list(ap.ap))


def build_kernel() -> bass.Bass:
    nc = bacc.Bacc(target_bir_lowering=False, debug=False)

    efh_d = nc.dram_tensor("efh", [BL, T, N], BF16, kind="ExternalInput")
    efl_d = nc.dram_tensor("efl", [BL, T, N], BF16, kind="ExternalInput")
    eoh_d = nc.dram_tensor("eoh", [BL, T, N], BF16, kind="ExternalInput")
    eol_d = nc.dram_tensor("eol", [BL, T, N], BF16, kind="ExternalInput")
    covq_d = nc.dram_tensor("covq", [BL, 5, T], BF16, kind="ExternalInput")
    cov_d = nc.dram_tensor("cov", [BL, T], F32, kind="ExternalInput")
    stmt_d = nc.dram_tensor("stmt", [BL, T], F32, kind="ExternalInput")
    mask_d = nc.dram_tensor("mask", [BL, T], F32, kind="ExternalInput")
    sT_d = nc.dram_tensor("sT", [N, BL], F32, kind="ExternalInput")
    wdt_d = nc.dram_tensor("wdect", [N, N], F32, kind="ExternalInput")
    bdec_d = nc.dram_tensor("bdec", [N], F32, kind="ExternalInput")
    vw_d = nc.dram_tensor("vw", [N], F32, kind="ExternalInput")
    wc3_d = nc.dram_tensor("wc3", [3, N], BF16, kind="ExternalInput")
    idb_d = nc.dram_tensor("identbf", [P, P], BF16, kind="ExternalInput")

    ct_d = nc.dram_tensor("ct", [BL, N], F32, kind="ExternalOutput")
    attn_d = nc.dram_tensor("attn", [BL, T], F32, kind="ExternalOutput")
    covout_d = nc.dram_tensor("covout", [BL, T], F32, kind="ExternalOutput")

    with tile.TileContext(nc) as tc, ExitStack() as ctx:
        consts = ctx.enter_context(tc.tile_pool(name="consts", bufs=1))
        combop = ctx.enter_context(tc.tile_pool(name="combop", bufs=1))
        covqp = ctx.enter_context(tc.tile_pool(name="covqp", bufs=2))
        efhp = ctx.enter_context(tc.tile_pool(name="efhp", bufs=3))
        eflp = ctx.enter_context(tc.tile_pool(name="eflp", bufs=3))
        eohp = ctx.enter_context(tc.tile_pool(name="eohp", bufs=5))
        eolp = ctx.enter_context(tc.tile_pool(name="eolp", bufs=5))
        epool = ctx.enter_context(tc.tile_pool(name="epool", bufs=3))
        small = ctx.enter_context(tc.tile_pool(name="small", bufs=2))
        ctp = ctx.enter_context(tc.tile_pool(name="ctp", bufs=2))
        px = ctx.enter_context(tc.tile_pool(name="px", bufs=3, space="PSUM"))
        paux = ctx.enter_context(tc.tile_pool(name="paux", bufs=1, space="PSUM"))

        # ---- constants -------------------------------------------------
        vwr = consts.tile([P, N], F32)
        nc.sync.dma_start(out=vwr[:], in_=_bc(vw_d[:], P))
        idb = consts.tile([P, P], BF16)
        nc.sync.dma_start(out=idb[:], in_=idb_d[:])
        sT_sb = consts.tile([P, KC, BL], F32)
        nc.sync.dma_start(out=sT_sb[:], in_=sT_d.rearrange("(c p) b -> p c b", p=P))
        bdec4 = consts.tile([BL, N], F32)
        nc.sync.dma_start(out=bdec4[:], in_=_bc(bdec_d[:], BL))
        neg_ones = consts.tile([1, P], F32)
        nc.vector.memset(neg_ones[:], -1.0)
        ones_row = consts.tile([1, P], F32)
        nc.vector.memset(ones_row[:], 1.0)
        ones_col = consts.tile([P, 1], F32)
        nc.vector.memset(ones_col[:], 1.0)

        # negS = -sum(|v_w|): a softmax shift that upper-bounds every score
        negs1 = consts.tile([1, 1], F32)
        nc.vector.tensor_reduce(out=negs1[:], in_=vwr[0:1, :], axis=AX.X,
                                op=ALU.add, apply_absolute_value=True)
        psb = paux.tile([P, 1], F32, tag="aux")
        nc.tensor.matmul(psb[:], lhsT=neg_ones[:], rhs=negs1[:])
        negS = consts.tile([P, 1], F32)
        nc.scalar.copy(negS[:], psb[:])

        # ---- dec = s_t_hat @ W_dec.T + b_dec  (fp32, exact) ------------
        psum_dec = px.tile([BL, N], F32, tag="x")
        for c in range(KC):
            wch = epool.tile([P, N], F32, tag="e")
            nc.sync.dma_start(out=wch[:], in_=wdt_d[c * P:(c + 1) * P, :])
            for h in range(2):
                nc.tensor.matmul(psum_dec[:, h * H:(h + 1) * H],
                                 lhsT=sT_sb[:, c, :],
                                 rhs=wch[:, h * H:(h + 1) * H],
                                 start=(c == 0), stop=(c == KC - 1))
        dec_sb = consts.tile([BL, N], F32)
        nc.vector.tensor_add(dec_sb[:], psum_dec[:], bdec4[:])
        # hi/lo bf16 split of dec for all 4 batch rows (partition base 0)
        dech4 = consts.tile([BL, N], BF16)
        nc.scalar.copy(dech4[:], dec_sb[:])
        decl4 = consts.tile([BL, N], BF16)
        nc.vector.affine_then_add(out=decl4[:], in0=dech4[:], in1=dec_sb[:],
                                  scale=-1.0, bias=0.0)

        # combo rhs tiles [5, N] bf16: rows = dech, decl, wch, wcl, wch
        combos = []
        for i in range(2):
            cb = combop.tile([5, N], BF16, tag=f"combo{i}")
            nc.sync.dma_start(out=cb[2:5, :], in_=wc3_d[:])
            combos.append(cb)

        # ---- main loop over local batch rows ---------------------------
        for b in range(BL):
            combo = combos[b % 2]
            # single-row moves via DMA (engines can't address partition base b)
            nc.gpsimd.dma_start(out=combo[0:1, :], in_=dech4[b:b + 1, :])
            nc.gpsimd.dma_start(out=combo[1:2, :], in_=decl4[b:b + 1, :])

            covq_sb = covqp.tile([5, T], BF16, tag="covq")
            nc.sync.dma_start(out=covq_sb[:], in_=covq_d[b])

            mask16 = small.tile([P, NJ], F32, tag="mask16")
            nc.sync.dma_start(out=mask16[:].rearrange("p (q a) -> p q a", a=A),
                              in_=mask_d[b].rearrange("(q p a) -> p q a", p=P, a=A))
            stmt16 = small.tile([P, NJ], F32, tag="stmt16")
            nc.sync.dma_start(out=stmt16[:].rearrange("p (q a) -> p q a", a=A),
                              in_=stmt_d[b].rearrange("(q p a) -> p q a", p=P, a=A))
            cov16 = small.tile([P, NJ], F32, tag="cov16")
            nc.sync.dma_start(out=cov16[:].rearrange("p (q a) -> p q a", a=A),
                              in_=cov_d[b].rearrange("(q p a) -> p q a", p=P, a=A))
            stmtm = small.tile([P, NJ], F32, tag="stmtm")
            nc.vector.tensor_mul(stmtm[:], stmt16[:], mask16[:])

            scores = small.tile([P, NJ], F32, tag="scores")
            eoh_tiles = []
            eol_tiles = []
            for q in range(NQ):
                efh_t = efhp.tile([P, A, N], BF16, tag="efh")
                nc.sync.dma_start(
                    out=efh_t[:],
                    in_=efh_d[b, QT * q:QT * (q + 1), :].rearrange("(p a) n -> p a n", p=P))
                efl_t = eflp.tile([P, A, N], BF16, tag="efl")
                nc.sync.dma_start(
                    out=efl_t[:],
                    in_=efl_d[b, QT * q:QT * (q + 1), :].rearrange("(p a) n -> p a n", p=P))
                eoh_t = eohp.tile([P, A, N], BF16, tag="eoh")
                nc.gpsimd.dma_start(
                    out=eoh_t[:],
                    in_=eoh_d[b, QT * q:QT * (q + 1), :].rearrange("(p a) n -> p a n", p=P))
                eol_t = eolp.tile([P, A, N], BF16, tag="eol")
                nc.gpsimd.dma_start(
                    out=eol_t[:],
                    in_=eol_d[b, QT * q:QT * (q + 1), :].rearrange("(p a) n -> p a n", p=P))
                eoh_tiles.append(eoh_t)
                eol_tiles.append(eol_t)

                for a in range(A):
                    j = q * A + a
                    pxt = px.tile([P, N], F32, tag="x")
                    off = QT * q + a
                    lsl = covq_sb[:, off: off + (P - 1) * A + 1: A]  # [5, 128]
                    for h in range(2):
                        hs = slice(h * H, (h + 1) * H)
                        nc.tensor.matmul(pxt[:, hs], lhsT=lsl, rhs=combo[:, hs],
                                         start=True, stop=False)
                        nc.tensor.matmul(pxt[:, hs], lhsT=idb[:],
                                         rhs=efh_t[:, a, hs], start=False, stop=False)
                        nc.tensor.matmul(pxt[:, hs], lhsT=idb[:],
                                         rhs=efl_t[:, a, hs], start=False, stop=True)
                    et = epool.tile([P, N], F32, tag="e")
                    nc.scalar.activation(et[:], pxt[:], AF.Tanh)
                    nc.vector.affine_mul_reduce(
                        out=et[:], accum_out=scores[:, j:j + 1],
                        in0=et[:], in1=vwr[:], scale=1.0, bias=0.0)

            # ---- softmax over all 2048 scores of this batch row --------
            es = small.tile([P, NJ], F32, tag="es")
            nc.scalar.activation(es[:], scores[:], AF.Exp, bias=negS[:])
            attn_u = small.tile([P, NJ], F32, tag="attn_u")
            s1 = small.tile([P, 1], F32, tag="s1")
            nc.vector.affine_mul_reduce(out=attn_u[:], accum_out=s1[:],
                                        in0=es[:], in1=mask16[:], scale=1.0, bias=0.0)
            ps1 = paux.tile([1, 1], F32, tag="aux")
            nc.tensor.matmul(ps1[:], lhsT=s1[:], rhs=ones_col[:])
            r1 = small.tile([1, 1], F32, tag="r1")
            nc.vector.reciprocal(r1[:], ps1[:])
            psr = paux.tile([P, 1], F32, tag="aux")
            nc.tensor.matmul(psr[:], lhsT=ones_row[:], rhs=r1[:])
            rb = small.tile([P, 1], F32, tag="rb")
            nc.scalar.copy(rb[:], psr[:])
            # attn = attn_u * r + stmt*mask  (one fused DVE op)
            attn_f = small.tile([P, NJ], F32, tag="attn_f")
            nc.vector.affine_then_add(out=attn_f[:], in0=attn_u[:], in1=stmtm[:],
                                      scale=rb[:], bias=0.0)
            covo = small.tile([P, NJ], F32, tag="covo")
            nc.vector.tensor_add(covo[:], cov16[:], attn_f[:])
            nc.sync.dma_start(
                out=attn_d[b].rearrange("(q p a) -> p q a", p=P, a=A),
                in_=attn_f[:].rearrange("p (q a) -> p q a", a=A))
            nc.sync.dma_start(
                out=covout_d[b].rearrange("(q p a) -> p q a", p=P, a=A),
                in_=covo[:].rearrange("p (q a) -> p q a", a=A))

            # ---- c_t = attn @ eo, bf16 hi/lo split ---------------------
            ah = small.tile([P, NJ], BF16, tag="ah")
            nc.scalar.copy(ah[:], attn_f[:])
            al = small.tile([P, NJ], BF16, tag="al")
            nc.vector.affine_then_add(out=al[:], in0=ah[:], in1=attn_f[:],
                                      scale=-1.0, bias=0.0)
            psct = paux.tile([1, N], F32, tag="aux")
            for j in range(NJ):
                q, a = divmod(j, A)
                for h in range(2):
                    sl = slice(h * H, (h + 1) * H)
                    nc.tensor.matmul(psct[:, sl], lhsT=ah[:, j:j + 1],
                                     rhs=eoh_tiles[q][:, a, sl],
                                     start=(j == 0), stop=False)
                    nc.tensor.matmul(psct[:, sl], lhsT=ah[:, j:j + 1],
                                     rhs=eol_tiles[q][:, a, sl],
                                     start=False, stop=False)
                    nc.tensor.matmul(psct[:, sl], lhsT=al[:, j:j + 1],
                                     rhs=eoh_tiles[q][:, a, sl],
                                     start=False, stop=(j == NJ - 1))
            ct_sb = ctp.tile([1, N], F32, tag="ctsb")
            nc.scalar.copy(ct_sb[:], psct[:])
            nc.sync.dma_start(out=ct_d[b:b + 1, :], in_=ct_sb[:])

    nc.compile()
    return nc


def _split_bf16(x):
    import ml_dtypes
    hi = x.astype(ml_dtypes.bfloat16)
    lo = (x - hi.astype(np.float32)).astype(ml_dtypes.bfloat16)
    return hi, lo


def prepare_in_maps(inputs):
    import ml_dtypes
    f32 = np.float32
    s = np.ascontiguousarray(np.asarray(inputs["s_t_hat"], f32))
    eo = np.ascontiguousarray(np.asarray(inputs["encoder_outputs"], f32))
    ef = np.ascontiguousarray(np.asarray(inputs["encoder_feature"], f32)).reshape(B, T, N)
    stmt = np.ascontiguousarray(np.asarray(inputs["stmt_feature"], f32))
    mask = np.ascontiguousarray(np.asarray(inputs["enc_padding_mask"], f32))
    cov = np.ascontiguousarray(np.asarray(inputs["coverage"], f32))
    W_dec = np.asarray(inputs["W_dec"], f32)
    b_dec = np.ascontiguousarray(np.asarray(inputs["b_dec"], f32))
    v_w = np.ascontiguousarray(np.asarray(inputs["v_w"], f32))
    W_c = np.asarray(inputs["W_c"], f32)

    efh, efl = _split_bf16(ef)
    eoh, eol = _split_bf16(eo)
    covh, covl = _split_bf16(cov)
    # covq rows: ones, ones, covh, covh, covl
    covq = np.empty((B, 5, T), dtype=ml_dtypes.bfloat16)
    covq[:, 0, :] = np.float32(1.0)
    covq[:, 1, :] = np.float32(1.0)
    covq[:, 2, :] = covh
    covq[:, 3, :] = covh
    covq[:, 4, :] = covl
    wch, wcl = _split_bf16(W_c)
    wc3 = np.stack([wch, wcl, wch], axis=0)       # [3, N]
    sT = np.ascontiguousarray(s.T)                # [N, B]
    wdt = np.ascontiguousarray(W_dec.T)           # [N, N]
    identbf = np.eye(P, dtype=ml_dtypes.bfloat16)

    in_maps = []
    for c in range(NCORES):
        bs = slice(c * BL, (c + 1) * BL)
        in_maps.append(dict(
            efh=np.ascontiguousarray(efh[bs]),
            efl=np.ascontiguousarray(efl[bs]),
            eoh=np.ascontiguousarray(eoh[bs]),
            eol=np.ascontiguousarray(eol[bs]),
            covq=np.ascontiguousarray(covq[bs]),
            cov=np.ascontiguousarray(cov[bs]),
            stmt=np.ascontiguousarray(stmt[bs]),
            mask=np.ascontiguousarray(mask[bs]),
            sT=np.ascontiguousarray(sT[:, bs]),
            wdect=wdt,
            bdec=b_dec,
            vw=v_w,
            wc3=wc3,
            identbf=identbf,
        ))
    return in_maps


def run(inputs, trace=False, **kw):
    nc = build_kernel()
    in_maps = prepare_in_maps(inputs)
    res = run_bass_kernel_spmd(nc, in_maps, list(range(NCORES)), trace=trace, **kw)
    c_t = np.concatenate([r["ct"] for r in res.results], axis=0)
    attn = np.concatenate([r["attn"] for r in res.results], axis=0)
    covout = np.concatenate([r["covout"] for r in res.results], axis=0)
    return (c_t, attn, covout), res


def kernel(**inputs):
    outs, _ = run(inputs, trace=False)
    return outs


# revision 4
# speedup vs baseline: 1.2696x; 1.2696x over previous
"""Trainium2 Bass kernel for nn_Attention_49340584296826 (pointer-generator
attention with coverage).

Computation per batch row b (B=32, T=2048, N=1024):
    dec   = s_t_hat @ W_dec.T + b_dec                       [N]
    x     = ef[b] + dec[None,:] + coverage[b][:,None]*W_c   [T, N]
    e     = tanh(x)
    s     = e @ v_w                                         [T]
    a     = exp(s - S) * mask;  a /= sum(a)   (S = sum|v_w| >= max s)
    a    += stmt * mask
    c_t   = a @ encoder_outputs[b]                          [N]
    cov'  = coverage + a

Sharding: data-parallel over batch, 4 rows per core on 8 cores.

Layout: t-chunks of 512 rows per DMA, arranged [128 partitions, 4, N] with
t = 512*q + 4*p + a (16KB contiguous per partition). Scores live as
[128, 16] with column j = 4*q + a.

Per 128-row sub-tile:
  PE  : psum = deccov via one K=5 bf16 matmul  (lhsT [1,1,covh,covh,covl] x
        rhs [dech,decl,wch,wcl,wch] -- hi/lo bf16 splits, ~1e-5 accurate)
  DVE : psum += ef  (tensor_add, fp32, in-place on PSUM)
  ACT : e = tanh(psum)
  DVE : affine_mul_reduce(e * v_w) -> scores column (fused mult+rowsum)
c_t runs as bf16 hi/lo matmuls (attn split on chip, eo split on host), and
batch b's c_t matmuls are interleaved into batch b+1's scores loop so the
PE never idles across the softmax chain (keeps HAM at full clock and
staggers eo buffer release).
"""
import numpy as np
from contextlib import ExitStack

import concourse.bass as bass
import concourse.bacc as bacc
import concourse.tile as tile
from concourse import mybir
from concourse.bass_utils import run_bass_kernel_spmd

F32 = mybir.dt.float32
BF16 = mybir.dt.bfloat16
AF = mybir.ActivationFunctionType
ALU = mybir.AluOpType
AX = mybir.AxisListType

B, T, N = 32, 2048, 1024
NCORES = 8
BL = B // NCORES          # 4 batch rows per core
P = 128
A = 4                     # t rows per partition per DMA chunk
QT = P * A                # 512 t per chunk
NQ = T // QT              # 4 chunks per batch
NJ = NQ * A               # 16 score columns per batch
H = 512                   # matmul free-dim split (one PSUM bank)
KC = N // P               # 8 contraction chunks for the dec matmul


def _bc(ap, parts):
    """Broadcast an AP across `parts` partitions (stride-0 partition dim)."""
    return bass.AP(tensor=ap.tensor, offset=ap.offset, ap=[[0, parts]] + list(ap.ap))


def build_kernel() -> bass.Bass:
    nc = bacc.Bacc(target_bir_lowering=False, debug=False)

    ef_d = nc.dram_tensor("ef", [BL, T, N], F32, kind="ExternalInput")
    eoh_d = nc.dram_tensor("eoh", [BL, T, N], BF16, kind="ExternalInput")
    eol_d = nc.dram_tensor("eol", [BL, T, N], BF16, kind="ExternalInput")
    covq_d = nc.dram_tensor("covq", [BL, 5, T], BF16, kind="ExternalInput")
    cov_d = nc.dram_tensor("cov", [BL, T], F32, kind="ExternalInput")
    stmt_d = nc.dram_tensor("stmt", [BL, T], F32, kind="ExternalInput")
    mask_d = nc.dram_tensor("mask", [BL, T], F32, kind="ExternalInput")
    sT_d = nc.dram_tensor("sT", [N, BL], F32, kind="ExternalInput")
    wdt_d = nc.dram_tensor("wdect", [N, N], F32, kind="ExternalInput")
    bdec_d = nc.dram_tensor("bdec", [N], F32, kind="ExternalInput")
    vw_d = nc.dram_tensor("vw", [N], F32, kind="ExternalInput")
    wc3_d = nc.dram_tensor("wc3", [3, N], BF16, kind="ExternalInput")

    ct_d = nc.dram_tensor("ct", [BL, N], F32, kind="ExternalOutput")
    attn_d = nc.dram_tensor("attn", [BL, T], F32, kind="ExternalOutput")
    covout_d = nc.dram_tensor("covout", [BL, T], F32, kind="ExternalOutput")

    with tile.TileContext(nc) as tc, ExitStack() as ctx:
        consts = ctx.enter_context(tc.tile_pool(name="consts", bufs=1))
        combop = ctx.enter_context(tc.tile_pool(name="combop", bufs=1))
        covqp = ctx.enter_context(tc.tile_pool(name="covqp", bufs=2))
        efp = ctx.enter_context(tc.tile_pool(name="efp", bufs=3))
        eohp = ctx.enter_context(tc.tile_pool(name="eohp", bufs=5))
        eolp = ctx.enter_context(tc.tile_pool(name="eolp", bufs=5))
        epool = ctx.enter_context(tc.tile_pool(name="epool", bufs=3))
        small = ctx.enter_context(tc.tile_pool(name="small", bufs=2))
        ctp = ctx.enter_context(tc.tile_pool(name="ctp", bufs=2))
        px = ctx.enter_context(tc.tile_pool(name="px", bufs=3, space="PSUM"))
        paux = ctx.enter_context(tc.tile_pool(name="paux", bufs=1, space="PSUM"))

        # ---- constants -------------------------------------------------
        vwr = consts.tile([P, N], F32)
        nc.sync.dma_start(out=vwr[:], in_=_bc(vw_d[:], P))
        sT_sb = consts.tile([P, KC, BL], F32)
        nc.sync.dma_start(out=sT_sb[:], in_=sT_d.rearrange("(c p) b -> p c b", p=P))
        bdec4 = consts.tile([BL, N], F32)
        nc.sync.dma_start(out=bdec4[:], in_=_bc(bdec_d[:], BL))
        neg_ones = consts.tile([1, P], F32)
        nc.vector.memset(neg_ones[:], -1.0)
        ones_row = consts.tile([1, P], F32)
        nc.vector.memset(ones_row[:], 1.0)
        ones_col = consts.tile([P, 1], F32)
        nc.vector.memset(ones_col[:], 1.0)

        # negS = -sum(|v_w|): a softmax shift that upper-bounds every score
        negs1 = consts.tile([1, 1], F32)
        nc.vector.tensor_reduce(out=negs1[:], in_=vwr[0:1, :], axis=AX.X,
                                op=ALU.add, apply_absolute_value=True)
        psb = paux.tile([P, 1], F32, tag="aux")
        nc.tensor.matmul(psb[:], lhsT=neg_ones[:], rhs=negs1[:])
        negS = consts.tile([P, 1], F32)
        nc.scalar.copy(negS[:], psb[:])

        # ---- dec = s_t_hat @ W_dec.T + b_dec  (fp32, exact) ------------
        psum_dec = px.tile([BL, N], F32, tag="x")
        for c in range(KC):
            wch = epool.tile([P, N], F32, tag="e")
            nc.sync.dma_start(out=wch[:], in_=wdt_d[c * P:(c + 1) * P, :])
            for h in range(2):
                nc.tensor.matmul(psum_dec[:, h * H:(h + 1) * H],
                                 lhsT=sT_sb[:, c, :],
                                 rhs=wch[:, h * H:(h + 1) * H],
                                 start=(c == 0), stop=(c == KC - 1))
        dec_sb = consts.tile([BL, N], F32)
        nc.vector.tensor_add(dec_sb[:], psum_dec[:], bdec4[:])
        # hi/lo bf16 split of dec for all 4 batch rows (partition base 0)
        dech4 = consts.tile([BL, N], BF16)
        nc.scalar.copy(dech4[:], dec_sb[:])
        decl4 = consts.tile([BL, N], BF16)
        nc.vector.affine_then_add(out=decl4[:], in0=dech4[:], in1=dec_sb[:],
                                  scale=-1.0, bias=0.0)

        # combo rhs tiles [5, N] bf16: rows = dech, decl, wch, wcl, wch
        combos = []
        for i in range(2):
            cb = combop.tile([5, N], BF16, tag=f"combo{i}")
            nc.sync.dma_start(out=cb[2:5, :], in_=wc3_d[:])
            combos.append(cb)

        # ---- main loop over local batch rows ---------------------------
        # pending = state of the previous batch row whose c_t matmuls are
        # interleaved into this row's scores loop (keeps the PE dense).
        pending = None

        def emit_ct(pend, js):
            ah_p, al_p, eoh_p, eol_p, psct_p = pend
            for j in js:
                q, a = divmod(j, A)
                for h in range(2):
                    sl = slice(h * H, (h + 1) * H)
                    nc.tensor.matmul(psct_p[:, sl], lhsT=ah_p[:, j:j + 1],
                                     rhs=eoh_p[q][:, a, sl],
                                     start=(j == 0), stop=False)
                    nc.tensor.matmul(psct_p[:, sl], lhsT=ah_p[:, j:j + 1],
                                     rhs=eol_p[q][:, a, sl],
                                     start=False, stop=False)
                    nc.tensor.matmul(psct_p[:, sl], lhsT=al_p[:, j:j + 1],
                                     rhs=eoh_p[q][:, a, sl],
                                     start=False, stop=(j == NJ - 1))

        def finish_ct(pend, b_prev):
            ct_sb = ctp.tile([1, N], F32, tag="ctsb")
            nc.scalar.copy(ct_sb[:], pend[4][:])
            nc.sync.dma_start(out=ct_d[b_prev:b_prev + 1, :], in_=ct_sb[:])

        for b in range(BL):
            combo = combos[b % 2]
            # single-row moves via DMA (engines can't address partition base b)
            nc.gpsimd.dma_start(out=combo[0:1, :], in_=dech4[b:b + 1, :])
            nc.gpsimd.dma_start(out=combo[1:2, :], in_=decl4[b:b + 1, :])

            covq_sb = covqp.tile([5, T], BF16, tag="covq")
            nc.sync.dma_start(out=covq_sb[:], in_=covq_d[b])

            mask16 = small.tile([P, NJ], F32, tag="mask16")
            nc.sync.dma_start(out=mask16[:].rearrange("p (q a) -> p q a", a=A),
                              in_=mask_d[b].rearrange("(q p a) -> p q a", p=P, a=A))
            stmt16 = small.tile([P, NJ], F32, tag="stmt16")
            nc.sync.dma_start(out=stmt16[:].rearrange("p (q a) -> p q a", a=A),
                              in_=stmt_d[b].rearrange("(q p a) -> p q a", p=P, a=A))
            cov16 = small.tile([P, NJ], F32, tag="cov16")
            nc.sync.dma_start(out=cov16[:].rearrange("p (q a) -> p q a", a=A),
                              in_=cov_d[b].rearrange("(q p a) -> p q a", p=P, a=A))
            stmtm = small.tile([P, NJ], F32, tag="stmtm")
            nc.vector.tensor_mul(stmtm[:], stmt16[:], mask16[:])

            scores = small.tile([P, NJ], F32, tag="scores")
            eoh_tiles = []
            eol_tiles = []
            for q in range(NQ):
                # alternate rings per chunk to keep both DGE paths loaded
                eng_a = nc.sync if q % 2 == 0 else nc.gpsimd
                eng_b = nc.gpsimd if q % 2 == 0 else nc.sync
                eft = efp.tile([P, A, N], F32, tag="ef")
                eng_a.dma_start(
                    out=eft[:],
                    in_=ef_d[b, QT * q:QT * (q + 1), :].rearrange("(p a) n -> p a n", p=P))
                eoh_t = eohp.tile([P, A, N], BF16, tag="eoh")
                eng_b.dma_start(
                    out=eoh_t[:],
                    in_=eoh_d[b, QT * q:QT * (q + 1), :].rearrange("(p a) n -> p a n", p=P))
                eol_t = eolp.tile([P, A, N], BF16, tag="eol")
                eng_b.dma_start(
                    out=eol_t[:],
                    in_=eol_d[b, QT * q:QT * (q + 1), :].rearrange("(p a) n -> p a n", p=P))
                eoh_tiles.append(eoh_t)
                eol_tiles.append(eol_t)

                for a in range(A):
                    j = q * A + a
                    pxt = px.tile([P, N], F32, tag="x")
                    off = QT * q + a
                    lsl = covq_sb[:, off: off + (P - 1) * A + 1: A]  # [5, 128]
                    for h in range(2):
                        hs = slice(h * H, (h + 1) * H)
                        nc.tensor.matmul(pxt[:, hs], lhsT=lsl, rhs=combo[:, hs],
                                         start=True, stop=True)
                    nc.vector.tensor_add(pxt[:], pxt[:], eft[:, a, :])
                    et = epool.tile([P, N], F32, tag="e")
                    nc.scalar.activation(et[:], pxt[:], AF.Tanh)
                    nc.vector.affine_mul_reduce(
                        out=et[:], accum_out=scores[:, j:j + 1],
                        in0=et[:], in1=vwr[:], scale=1.0, bias=0.0)

                # previous batch's c_t for 4 columns (releases its eo tiles)
                if pending is not None:
                    emit_ct(pending, range(q * A, (q + 1) * A))
                    if q == NQ - 1:
                        finish_ct(pending, b - 1)
                        pending = None

            # ---- softmax over all 2048 scores of this batch row --------
            es = small.tile([P, NJ], F32, tag="es")
            nc.scalar.activation(es[:], scores[:], AF.Exp, bias=negS[:])
            attn_u = small.tile([P, NJ], F32, tag="attn_u")
            s1 = small.tile([P, 1], F32, tag="s1")
            nc.vector.affine_mul_reduce(out=attn_u[:], accum_out=s1[:],
                                        in0=es[:], in1=mask16[:], scale=1.0, bias=0.0)
            ps1 = paux.tile([1, 1], F32, tag="aux")
            nc.tensor.matmul(ps1[:], lhsT=s1[:], rhs=ones_col[:])
            r1 = small.tile([1, 1], F32, tag="r1")
            nc.vector.reciprocal(r1[:], ps1[:])
            psr = paux.tile([P, 1], F32, tag="aux")
            nc.tensor.matmul(psr[:], lhsT=ones_row[:], rhs=r1[:])
            rb = small.tile([P, 1], F32, tag="rb")
            nc.scalar.copy(rb[:], psr[:])
            # attn = attn_u * r + stmt*mask  (one fused DVE op)
            attn_f = small.tile([P, NJ], F32, tag="attn_f")
            nc.vector.affine_then_add(out=attn_f[:], in0=attn_u[:], in1=stmtm[:],
                                      scale=rb[:], bias=0.0)
            covo = small.tile([P, NJ], F32, tag="covo")
            nc.vector.tensor_add(covo[:], cov16[:], attn_f[:])
            nc.sync.dma_start(
                out=attn_d[b].rearrange("(q p a) -> p q a", p=P, a=A),
                in_=attn_f[:].rearrange("p (q a) -> p q a", a=A))
            nc.sync.dma_start(
                out=covout_d[b].rearrange("(q p a) -> p q a", p=P, a=A),
                in_=covo[:].rearrange("p (q a) -> p q a", a=A))

            # attn hi/lo split for the c_t matmuls
            ah = small.tile([P, NJ], BF16, tag="ah")
            nc.scalar.copy(ah[:], attn_f[:])
            al = small.tile([P, NJ], BF16, tag="al")
            nc.vector.affine_then_add(out=al[:], in0=ah[:], in1=attn_f[:],
                                      scale=-1.0, bias=0.0)
            psct = paux.tile([1, N], F32, tag="aux")
            pending = (ah, al, eoh_tiles, eol_tiles, psct)

        # drain the last batch row's c_t
        emit_ct(pending, range(NJ))
        finish_ct(pending, BL - 1)

    nc.compile()
    return nc


def _split_bf16(x):
    import ml_dtypes
    hi = x.astype(ml_dtypes.bfloat16)
    lo = (x - hi.astype(np.float32)).astype(ml_dtypes.bfloat16)
    return hi, lo


def prepare_in_maps(inputs):
    import ml_dtypes
    f32 = np.float32
    s = np.ascontiguousarray(np.asarray(inputs["s_t_hat"], f32))
    eo = np.ascontiguousarray(np.asarray(inputs["encoder_outputs"], f32))
    ef = np.ascontiguousarray(np.asarray(inputs["encoder_feature"], f32)).reshape(B, T, N)
    stmt = np.ascontiguousarray(np.asarray(inputs["stmt_feature"], f32))
    mask = np.ascontiguousarray(np.asarray(inputs["enc_padding_mask"], f32))
    cov = np.ascontiguousarray(np.asarray(inputs["coverage"], f32))
    W_dec = np.asarray(inputs["W_dec"], f32)
    b_dec = np.ascontiguousarray(np.asarray(inputs["b_dec"], f32))
    v_w = np.ascontiguousarray(np.asarray(inputs["v_w"], f32))
    W_c = np.asarray(inputs["W_c"], f32)

    eoh, eol = _split_bf16(eo)
    covh, covl = _split_bf16(cov)
    # covq rows: ones, ones, covh, covh, covl
    covq = np.empty((B, 5, T), dtype=ml_dtypes.bfloat16)
    covq[:, 0, :] = np.float32(1.0)
    covq[:, 1, :] = np.float32(1.0)
    covq[:, 2, :] = covh
    covq[:, 3, :] = covh
    covq[:, 4, :] = covl
    wch, wcl = _split_bf16(W_c)
    wc3 = np.stack([wch, wcl, wch], axis=0)       # [3, N]
    sT = np.ascontiguousarray(s.T)                # [N, B]
    wdt = np.ascontiguousarray(W_dec.T)           # [N, N]

    in_maps = []
    for c in range(NCORES):
        bs = slice(c * BL, (c + 1) * BL)
        in_maps.append(dict(
            ef=np.ascontiguousarray(ef[bs]),
            eoh=np.ascontiguousarray(eoh[bs]),
            eol=np.ascontiguousarray(eol[bs]),
            covq=np.ascontiguousarray(covq[bs]),
            cov=np.ascontiguousarray(cov[bs]),
            stmt=np.ascontiguousarray(stmt[bs]),
            mask=np.ascontiguousarray(mask[bs]),
            sT=np.ascontiguousarray(sT[:, bs]),
            wdect=wdt,
            bdec=b_dec,
            vw=v_w,
            wc3=wc3,
        ))
    return in_maps


def run(inputs, trace=False, **kw):
    nc = build_kernel()
    in_maps = prepare_in_maps(inputs)
    res = run_bass_kernel_spmd(nc, in_maps, list(range(NCORES)), trace=trace, **kw)
    c_t = np.concatenate([r["ct"] for r in res.results], axis=0)
    attn = np.concatenate([r["attn"] for r in res.results], axis=0)
    covout = np.concatenate([r["covout"] for r in res.results], axis=0)
    return (c_t, attn, covout), res


def kernel(**inputs):
    outs, _ = run(inputs, trace=False)
    return outs
